# revision 1
# baseline (speedup 1.0000x reference)
"""CostVolume2D Trainium2 Bass kernel.

cost[n,d,h,w] = mean_c l[n,c,h,w] * r[n,c,h,w-d]  (0 for w < d)
N=8, C=32, H=256, W=512, D=64.  Data-parallel over batch: core i handles n=i.

Per-core algorithm (v-partition band correlation):
  For each row h and each v-block pair (2 blocks of 128 v each):
    M[v, w] = sum_c r[c,v] * l[c,w]  via TensorE matmuls (bf16, fp32 acc):
      stationary = r[c, wb:wb+128], moving = l[c, wb:wb+192]
    strip[p, d] = M[wb+p, n=p+d] = cost[d, h, wb+p+d]
  Strip extraction (a per-partition shear) rides a DRAM scratch round trip:
  the band tile [128, 384] is stored with flat-DRAM stride 447 per row
  (scratch[447*p + n] = band[p, n]), so strips become a rect gather
  ([[448,128],[192,2],[1,64]]).  Strips are transposed with the DMA xbar
  ([128,128] bf16) giving T[(k,d), p] = cost[d, h, wb_k+p+d], stored as
  contiguous 128-wide w-runs at flat stride HP*WOP+1.  The output tensor is
  padded [64, 257, 576] bf16: run-shift spill and w<d zero-store spill land
  in the padding, sliced off (and cast to f32) on the host.
"""

import numpy as np

_CACHE = {}

C, H, W, D = 32, 256, 512, 64
N_CORES = 8
WP = W + 64      # padded moving width
HP = H + 1       # padded out rows (absorbs h=0 zero-store spill)
WOP = W + 64     # padded out cols (absorbs w-run shift spill)
STILE = 127 * 447 + 384   # scratch elems per (h, wpair) tile


def _build(h_rows=H):
    import concourse.tile as tile
    from concourse import bacc, mybir
    from concourse.ap import AP

    f32 = mybir.dt.float32
    bf16 = mybir.dt.bfloat16

    nc = bacc.Bacc("TRN2", target_bir_lowering=False, debug=False)
    l_d = nc.dram_tensor("l", [C, h_rows, W], f32, kind="ExternalInput")
    r_d = nc.dram_tensor("r", [C, h_rows, W], f32, kind="ExternalInput")
    o_d = nc.dram_tensor("o", [1, D * (h_rows + 1) * WOP], bf16,
                         kind="ExternalOutput")
    scr = nc.dram_tensor("scr", [1, 2 * h_rows * STILE], bf16, kind="Internal")
    HPWOP = (h_rows + 1) * WOP

    with tile.TileContext(nc) as tc:
        with (
            tc.tile_pool(name="io", bufs=4) as io_pool,
            tc.tile_pool(name="band", bufs=6) as band_pool,
            tc.tile_pool(name="xp", bufs=6) as xp_pool,
            tc.tile_pool(name="const", bufs=1) as const_pool,
            tc.tile_pool(name="psum", bufs=4, space="PSUM") as psum_pool,
        ):
            zero64 = const_pool.tile([64, 64], bf16)
            nc.gpsimd.memset(zero64[:], 0.0)

            for h in range(h_rows):
                lt = io_pool.tile([C, WP], bf16, tag="lt")
                nc.gpsimd.dma_start(lt[:, 0:W], l_d[:, h, :])  # f32->bf16 cast
                nc.gpsimd.memset(lt[:, W:WP], 0.0)
                rt = io_pool.tile([C, W], bf16, tag="rt")
                nc.gpsimd.dma_start(rt[:], r_d[:, h, :])

                # zeros for w < d of this row (spill -> previous row's pad)
                zdst = AP(o_d.ap().tensor, (1 + h) * WOP - 64,
                          [[HPWOP + 1, 64], [1, 64]])
                nc.sync.dma_start(zdst, zero64[:])

                for wpair in range(2):
                    psum2 = psum_pool.tile([128, 384], f32, tag="ps")
                    for k in range(2):
                        wb = (2 * wpair + k) * 128
                        nc.tensor.matmul(
                            psum2[:, 192 * k:192 * k + 192],
                            rt[:, wb:wb + 128],
                            lt[:, wb:wb + 192],
                            start=True, stop=True,
                        )
                    band = band_pool.tile([128, 384], bf16, tag="band")
                    if wpair == 0:
                        nc.vector.tensor_scalar_mul(band[:], psum2[:], 1.0 / C)
                    else:
                        nc.scalar.mul(band[:], psum2[:], 1.0 / C)

                    # sheared scratch write: scr[447*p + n] = band[p, n]
                    t = 2 * h + wpair
                    sw = AP(scr.ap().tensor, t * STILE, [[447, 128], [1, 384]])
                    (nc.sync if wpair == 0 else nc.scalar).dma_start(sw, band[:])

                    # rect strips gather: strips[p, (k,d)] = scr[448p+192k+d]
                    strips = xp_pool.tile([128, 128], bf16, tag="strips")
                    rd = AP(scr.ap().tensor, t * STILE,
                            [[448, 128], [192, 2], [1, 64]])
                    (nc.scalar if wpair == 0 else nc.sync).dma_start(strips[:], rd)

                    xt = xp_pool.tile([128, 128], bf16, tag="xt")
                    nc.sync.dma_start(xt[:], strips[:], transpose=True)

                    # store halves: out[d, 1+h, wb + d + p], p in [0,128)
                    for k in range(2):
                        sdst = AP(o_d.ap().tensor,
                                  (1 + h) * WOP + (2 * wpair + k) * 128,
                                  [[HPWOP + 1, 64], [1, 128]])
                        eng = nc.sync if k == 0 else nc.scalar
                        eng.dma_start(sdst, xt[64 * k:64 * k + 64, :])
    nc.compile()
    return nc


def _get_nc(h_rows=H):
    if h_rows not in _CACHE:
        _CACHE[h_rows] = _build(h_rows)
    return _CACHE[h_rows]


def kernel(l_fmap, r_fmap, use_naive, max_disp):
    from concourse.bass_utils import run_bass_kernel_spmd

    l_fmap = np.asarray(l_fmap, dtype=np.float32)
    r_fmap = np.asarray(r_fmap, dtype=np.float32)
    assert int(max_disp) == D, f"kernel hardcoded for max_disp={D}"
    n, c, h, w = l_fmap.shape
    assert (n, c, h, w) == (N_CORES, C, H, W)

    nc = _get_nc(H)
    in_maps = [
        {"l": np.ascontiguousarray(l_fmap[i]), "r": np.ascontiguousarray(r_fmap[i])}
        for i in range(N_CORES)
    ]
    res = run_bass_kernel_spmd(nc, in_maps, core_ids=list(range(N_CORES)))
    out = np.stack([
        np.asarray(res.results[i]["o"]).reshape(D, HP, WOP)[:, 1:, 0:W]
        for i in range(N_CORES)
    ])
    return out.astype(np.float32)

